# revision 18
# baseline (speedup 1.0000x reference)
"""AttentionDownSample Trainium2 kernel (8 NeuronCores, data-parallel over batch).

Reference computation per batch b (fm [C=128, H=256, W=256], d=2):
  window tokens x_t[c, oh, ow] = fm[c, 2*oh+dy, 2*ow+dx], t = dy*2+dx
  q  = mean_t x_t                      -> q_proj = (q @ Wq) * Cr^-0.5
  k_t = x_t @ Wk
  l_t = q_proj . k_t  (over Cr=32)
  a_t = softmax_t(l_t)
  out[c, oh, ow] = sum_t a_t * x_t[c, oh, ow]

Device strategy (per core, one batch):
  - DMA fm rows into SBUF in C-partition layout (even/odd row planes).
  - TensorE: lhsT = pixel slab slice [C, 128 positions] (stationary), rhs =
    [Wk | Wq_eff] -> k_t and accumulated q_proj land position-major in PSUM.
  - DVE/ACT: tiny per-position mul/reduce/exp/recip -> softmax weights w
    [128 pos, 4].
  - TensorE: transpose w (4 x 128 per row) and broadcast each w_t row across
    128 partitions with a K=1 rank-1 matmul (ones ⊗ w_t).
  - DVE: out = sum_t x_t * u_t  (4 mul + 3 add), DMA out.
"""

import os
import sys

sys.path.insert(0, "/opt/trn_rl_repo")

import numpy as np

B, C, H, W = 8, 128, 256, 256
CR = 32
NH, NW = H // 2, W // 2
N_CORES = 8
ROWS_PER_CHUNK = 4  # output rows (oh) per chunk


def build_bass(n_oh_rows=NH):
    import concourse.bass as bass
    import concourse.mybir as mybir
    from concourse import bacc, tile

    f32 = mybir.dt.float32
    nc = bacc.Bacc()

    n_h = n_oh_rows * 2
    fm = nc.declare_dram_parameter("fm", [C, n_h, W], f32, isOutput=False)
    wqk = nc.declare_dram_parameter("wqk", [C, 2 * CR], f32, isOutput=False)
    ident = nc.declare_dram_parameter("ident", [128, 128], f32, isOutput=False)
    # sel[k, t, m] = 1.0 iff k == t ; lhsT selector for row-broadcast matmuls
    sel = nc.declare_dram_parameter("sel", [4, 4, 128], f32, isOutput=False)
    out = nc.declare_dram_parameter("out", [C, n_oh_rows, NW], f32, isOutput=True)

    n_chunks = n_oh_rows // ROWS_PER_CHUNK
    R = ROWS_PER_CHUNK
    bf16 = mybir.dt.bfloat16

    with tile.TileContext(nc) as tc:
        with (
            tc.tile_pool(name="const", bufs=1) as cpool,
            tc.tile_pool(name="io", bufs=3) as iopool,
            tc.tile_pool(name="work", bufs=2) as wpool,
            tc.tile_pool(name="psum", bufs=1, space="PSUM") as ppool,
        ):
            wqk_s = cpool.tile([C, 2 * CR], bf16)
            nc.gpsimd.dma_start(wqk_s[:], wqk[:])
            ident_s = cpool.tile([128, 128], f32)
            nc.sync.dma_start(ident_s[:], ident[:])
            sel_s = cpool.tile([4, 4, 128], f32)
            nc.sync.dma_start(sel_s[:], sel[:])

            for ch in range(n_chunks):
                h0 = ch * 2 * R
                # even / odd source rows for this chunk, cast f32->bf16 in DMA
                fm_e = iopool.tile([C, R, W], bf16, tag="fm_e")
                fm_o = iopool.tile([C, R, W], bf16, tag="fm_o")
                nc.gpsimd.dma_start(fm_e[:], fm[:, h0 : h0 + 2 * R : 2, :])
                nc.gpsimd.dma_start(fm_o[:], fm[:, h0 + 1 : h0 + 2 * R : 2, :])

                # ---- projections: k_t and q_proj, position-major ----
                # psum_k[pos, r, t, cr]; psum_q[pos, r, cr]
                psum_k = ppool.tile([128, R, 4, CR], f32, tag="pk")
                psum_q = ppool.tile([128, R, CR], f32, tag="pq")
                for r in range(R):
                    for t in range(4):
                        dy, dx = t // 2, t % 2
                        src = fm_e if dy == 0 else fm_o
                        xsl = src[:, r, dx::2]  # [128, 128] strided
                        nc.tensor.matmul(
                            psum_k[:, r, t, :], xsl, wqk_s[:, 0:CR],
                            start=True, stop=True,
                        )
                        nc.tensor.matmul(
                            psum_q[:, r, :], xsl, wqk_s[:, CR : 2 * CR],
                            start=(t == 0), stop=(t == 3),
                        )

                # ---- logits + softmax (position-major, DVE/ACT) ----
                qs = wpool.tile([128, R, CR], f32, tag="qs")
                nc.scalar.copy(qs[:], psum_q[:])
                prod = wpool.tile([128, R, 4, CR], f32, tag="prod")
                _q = qs[:]
                qs_b = bass.AP(_q.tensor, _q.offset, _q.ap[:2] + [[0, 4]] + _q.ap[2:])
                nc.vector.tensor_tensor(
                    prod[:], psum_k[:], qs_b, mybir.AluOpType.mult
                )
                logit = wpool.tile([128, R, 4], f32, tag="logit")
                nc.vector.tensor_reduce(
                    logit[:], prod[:], mybir.AxisListType.X, mybir.AluOpType.add
                )
                el = wpool.tile([128, R, 4], f32, tag="el")
                nc.scalar.activation(
                    el[:], logit[:], mybir.ActivationFunctionType.Exp
                )
                zsum = wpool.tile([128, R], f32, tag="zsum")
                nc.vector.tensor_reduce(
                    zsum[:], el[:], mybir.AxisListType.X, mybir.AluOpType.add
                )
                rz = wpool.tile([128, R], f32, tag="rz")
                nc.vector.reciprocal(rz[:], zsum[:])
                wgt = wpool.tile([128, R, 4], f32, tag="wgt")
                _rz = rz[:]
                rz_b = bass.AP(_rz.tensor, _rz.offset, _rz.ap + [[0, 4]])
                nc.vector.tensor_tensor(
                    wgt[:], el[:], rz_b, mybir.AluOpType.mult
                )

                # ---- transpose w and broadcast across partitions ----
                psum_wt = ppool.tile([4, R, 128], f32, tag="pwt")
                for r in range(R):
                    nc.tensor.transpose(
                        psum_wt[:, r, :], wgt[:, r, :], ident_s[:]
                    )
                wts = wpool.tile([4, R, 128], f32, tag="wts")
                nc.scalar.copy(wts[:], psum_wt[:])

                psum_u = ppool.tile([128, 4, R, 128], f32, tag="pu")
                for t in range(4):
                    nc.tensor.matmul(
                        psum_u[:, t], sel_s[:, t, :],
                        wts[:].rearrange("k r m -> k (r m)"),
                        start=True, stop=True,
                    )

                # copy PSUM->SBUF bf16 with interleaving dest AP:
                # ui[p, dy, r, 2*ow+dx] = w_{2dy+dx}[r, ow]
                ui = wpool.tile([128, 2, R, W], bf16, tag="ui")
                for t in range(4):
                    dy, dx = t // 2, t % 2
                    nc.scalar.copy(ui[:, dy, :, dx::2], psum_u[:, t])

                # ---- weighted sum: all contiguous bf16 (2x DVE mode) ----
                ve = wpool.tile([128, R, W], bf16, tag="ve")
                vo = wpool.tile([128, R, W], bf16, tag="vo")
                nc.vector.tensor_tensor(
                    ve[:], fm_e[:], ui[:, 0], mybir.AluOpType.mult
                )
                nc.vector.tensor_tensor(
                    vo[:], fm_o[:], ui[:, 1], mybir.AluOpType.mult
                )
                vs = wpool.tile([128, R, W], bf16, tag="vs")
                nc.vector.tensor_tensor(vs[:], ve[:], vo[:], mybir.AluOpType.add)
                acc = wpool.tile([128, R, 128], f32, tag="acc")
                nc.vector.tensor_tensor(
                    acc[:], vs[:, :, 0::2], vs[:, :, 1::2], mybir.AluOpType.add
                )

                nc.sync.dma_start(out[:, ch * R : (ch + 1) * R, :], acc[:])

    nc.compile()
    return nc


_NC_CACHE = {}


def _get_nc(n_oh_rows=NH):
    if n_oh_rows not in _NC_CACHE:
        _NC_CACHE[n_oh_rows] = build_bass(n_oh_rows)
    return _NC_CACHE[n_oh_rows]


def _make_in_maps(fm, Wq, Wk):
    wq_eff = (Wq.astype(np.float64) * (CR ** -0.5) / 4.0).astype(np.float32)
    wqk = np.concatenate([Wk.astype(np.float32), wq_eff], axis=1)
    wqk = np.ascontiguousarray(wqk)
    ident = np.eye(128, dtype=np.float32)
    sel = np.zeros((4, 4, 128), dtype=np.float32)
    for t in range(4):
        sel[t, t, :] = 1.0
    return [
        {
            "fm": np.ascontiguousarray(fm[i]),
            "wqk": wqk,
            "ident": ident,
            "sel": sel,
        }
        for i in range(fm.shape[0])
    ]


def kernel(fm, Wq, Wk):
    from concourse.bass_utils import run_bass_kernel_spmd

    fm = np.asarray(fm, dtype=np.float32)
    Wq = np.asarray(Wq, dtype=np.float32)
    Wk = np.asarray(Wk, dtype=np.float32)

    nc = _get_nc()
    in_maps = _make_in_maps(fm, Wq, Wk)
    res = run_bass_kernel_spmd(nc, in_maps, core_ids=list(range(N_CORES)))
    outs = [np.asarray(res.results[i]["out"]) for i in range(N_CORES)]
    return np.stack(outs, axis=0)


# revision 20
# speedup vs baseline: 2.4834x; 2.4834x over previous
"""AttentionDownSample Trainium2 kernel (8 NeuronCores, data-parallel over batch).

Reference computation per batch b (fm [C=128, H=256, W=256], d=2):
  window tokens x_t[c, oh, ow] = fm[c, 2*oh+dy, 2*ow+dx], t = dy*2+dx
  q  = mean_t x_t                      -> q_proj = (q @ Wq) * Cr^-0.5
  k_t = x_t @ Wk
  l_t = q_proj . k_t  (over Cr=32)
  a_t = softmax_t(l_t)
  out[c, oh, ow] = sum_t a_t * x_t[c, oh, ow]

Device strategy (per core, one batch):
  - DMA fm rows into SBUF in C-partition layout (even/odd row planes).
  - TensorE: lhsT = pixel slab slice [C, 128 positions] (stationary), rhs =
    [Wk | Wq_eff] -> k_t and accumulated q_proj land position-major in PSUM.
  - DVE/ACT: tiny per-position mul/reduce/exp/recip -> softmax weights w
    [128 pos, 4].
  - TensorE: transpose w (4 x 128 per row) and broadcast each w_t row across
    128 partitions with a K=1 rank-1 matmul (ones ⊗ w_t).
  - DVE: out = sum_t x_t * u_t  (4 mul + 3 add), DMA out.
"""

import os
import sys

sys.path.insert(0, "/opt/trn_rl_repo")

import numpy as np

B, C, H, W = 8, 128, 256, 256
CR = 32
NH, NW = H // 2, W // 2
N_CORES = 8
ROWS_PER_CHUNK = 4  # output rows (oh) per chunk


def build_bass(n_oh_rows=NH):
    import concourse.bass as bass
    import concourse.mybir as mybir
    from concourse import bacc, tile

    f32 = mybir.dt.float32
    nc = bacc.Bacc()

    n_h = n_oh_rows * 2
    fm = nc.declare_dram_parameter("fm", [C, n_h, W], f32, isOutput=False)
    wqk = nc.declare_dram_parameter("wqk", [C, 2 * CR], f32, isOutput=False)
    ident = nc.declare_dram_parameter("ident", [128, 128], f32, isOutput=False)
    # sel[k, t, m] = 1.0 iff k == t ; lhsT selector for row-broadcast matmuls
    sel = nc.declare_dram_parameter("sel", [4, 4, 128], f32, isOutput=False)
    out = nc.declare_dram_parameter("out", [C, n_oh_rows, NW], f32, isOutput=True)

    n_chunks = n_oh_rows // ROWS_PER_CHUNK
    R = ROWS_PER_CHUNK
    bf16 = mybir.dt.bfloat16

    with tile.TileContext(nc) as tc:
        with (
            tc.tile_pool(name="const", bufs=1) as cpool,
            tc.tile_pool(name="io", bufs=3) as iopool,
            tc.tile_pool(name="work", bufs=2) as wpool,
            tc.tile_pool(name="psum", bufs=1, space="PSUM") as ppool,
        ):
            wqk_s = cpool.tile([C, 2 * CR], bf16)
            nc.gpsimd.dma_start(wqk_s[:], wqk[:])
            ident_s = cpool.tile([128, 128], bf16)
            nc.gpsimd.dma_start(ident_s[:], ident[:])
            sel_s = cpool.tile([4, 4, 128], bf16)
            nc.gpsimd.dma_start(sel_s[:], sel[:])

            for ch in range(n_chunks):
                h0 = ch * 2 * R
                # even / odd source rows for this chunk, cast f32->bf16 in DMA
                fm_e = iopool.tile([C, R, W], bf16, tag="fm_e")
                fm_o = iopool.tile([C, R, W], bf16, tag="fm_o")
                nc.gpsimd.dma_start(fm_e[:], fm[:, h0 : h0 + 2 * R : 2, :])
                nc.gpsimd.dma_start(fm_o[:], fm[:, h0 + 1 : h0 + 2 * R : 2, :])

                # ---- projections: k_t and q_proj, position-major ----
                # psum_k[pos, r, t, cr]; psum_q[pos, r, cr]
                psum_k = ppool.tile([128, R, 4, CR], f32, tag="pk")
                psum_q = ppool.tile([128, R, CR], f32, tag="pq")
                for r in range(R):
                    for t in range(4):
                        dy, dx = t // 2, t % 2
                        src = fm_e if dy == 0 else fm_o
                        xsl = src[:, r, dx::2]  # [128, 128] strided
                        nc.tensor.matmul(
                            psum_k[:, r, t, :], xsl, wqk_s[:, 0:CR],
                            start=True, stop=True,
                        )
                        nc.tensor.matmul(
                            psum_q[:, r, :], xsl, wqk_s[:, CR : 2 * CR],
                            start=(t == 0), stop=(t == 3),
                        )

                # ---- logits + softmax (position-major, DVE/ACT) ----
                qs = wpool.tile([128, R, CR], f32, tag="qs")
                nc.scalar.copy(qs[:], psum_q[:])
                prod = wpool.tile([128, R, 4, CR], f32, tag="prod")
                _q = qs[:]
                qs_b = bass.AP(_q.tensor, _q.offset, _q.ap[:2] + [[0, 4]] + _q.ap[2:])
                nc.vector.tensor_tensor(
                    prod[:], psum_k[:], qs_b, mybir.AluOpType.mult
                )
                logit = wpool.tile([128, R, 4], f32, tag="logit")
                nc.vector.tensor_reduce(
                    logit[:], prod[:], mybir.AxisListType.X, mybir.AluOpType.add
                )
                el = wpool.tile([128, R, 4], f32, tag="el")
                nc.scalar.activation(
                    el[:], logit[:], mybir.ActivationFunctionType.Exp
                )
                zsum = wpool.tile([128, R], f32, tag="zsum")
                nc.vector.tensor_reduce(
                    zsum[:], el[:], mybir.AxisListType.X, mybir.AluOpType.add
                )
                rz = wpool.tile([128, R], f32, tag="rz")
                nc.vector.reciprocal(rz[:], zsum[:])
                wgt = wpool.tile([128, R, 4], bf16, tag="wgt")
                _rz = rz[:]
                rz_b = bass.AP(_rz.tensor, _rz.offset, _rz.ap + [[0, 4]])
                nc.vector.tensor_tensor(
                    wgt[:], el[:], rz_b, mybir.AluOpType.mult
                )

                # ---- transpose w and broadcast across partitions ----
                psum_wt = ppool.tile([4, R, 128], bf16, tag="pwt")
                for r in range(R):
                    nc.tensor.transpose(
                        psum_wt[:, r, :], wgt[:, r, :], ident_s[:]
                    )
                wts = wpool.tile([4, R, 128], bf16, tag="wts")
                nc.scalar.copy(wts[:], psum_wt[:])

                psum_u = ppool.tile([128, 4, R, 128], f32, tag="pu")
                for t in range(4):
                    nc.tensor.matmul(
                        psum_u[:, t], sel_s[:, t, :],
                        wts[:].rearrange("k r m -> k (r m)"),
                        start=True, stop=True,
                    )

                # copy PSUM->SBUF bf16 with interleaving dest AP:
                # ui[p, dy, r, 2*ow+dx] = w_{2dy+dx}[r, ow]
                ui = wpool.tile([128, 2, R, W], bf16, tag="ui")
                for t in range(4):
                    dy, dx = t // 2, t % 2
                    nc.scalar.copy(ui[:, dy, :, dx::2], psum_u[:, t])

                # ---- weighted sum: all contiguous bf16 (2x DVE mode) ----
                ve = wpool.tile([128, R, W], bf16, tag="ve")
                vo = wpool.tile([128, R, W], bf16, tag="vo")
                nc.vector.tensor_tensor(
                    ve[:], fm_e[:], ui[:, 0], mybir.AluOpType.mult
                )
                nc.vector.tensor_tensor(
                    vo[:], fm_o[:], ui[:, 1], mybir.AluOpType.mult
                )
                vs = wpool.tile([128, R, W], bf16, tag="vs")
                nc.vector.tensor_tensor(vs[:], ve[:], vo[:], mybir.AluOpType.add)
                acc = wpool.tile([128, R, 128], f32, tag="acc")
                nc.vector.tensor_tensor(
                    acc[:], vs[:, :, 0::2], vs[:, :, 1::2], mybir.AluOpType.add
                )

                nc.sync.dma_start(out[:, ch * R : (ch + 1) * R, :], acc[:])

    nc.compile()
    return nc


_NC_CACHE = {}


def _get_nc(n_oh_rows=NH):
    if n_oh_rows not in _NC_CACHE:
        _NC_CACHE[n_oh_rows] = build_bass(n_oh_rows)
    return _NC_CACHE[n_oh_rows]


def _make_in_maps(fm, Wq, Wk):
    wq_eff = (Wq.astype(np.float64) * (CR ** -0.5) / 4.0).astype(np.float32)
    wqk = np.concatenate([Wk.astype(np.float32), wq_eff], axis=1)
    wqk = np.ascontiguousarray(wqk)
    ident = np.eye(128, dtype=np.float32)
    sel = np.zeros((4, 4, 128), dtype=np.float32)
    for t in range(4):
        sel[t, t, :] = 1.0
    return [
        {
            "fm": np.ascontiguousarray(fm[i]),
            "wqk": wqk,
            "ident": ident,
            "sel": sel,
        }
        for i in range(fm.shape[0])
    ]


def kernel(fm, Wq, Wk):
    from concourse.bass_utils import run_bass_kernel_spmd

    fm = np.asarray(fm, dtype=np.float32)
    Wq = np.asarray(Wq, dtype=np.float32)
    Wk = np.asarray(Wk, dtype=np.float32)

    nc = _get_nc()
    in_maps = _make_in_maps(fm, Wq, Wk)
    res = run_bass_kernel_spmd(nc, in_maps, core_ids=list(range(N_CORES)))
    outs = [np.asarray(res.results[i]["out"]) for i in range(N_CORES)]
    return np.stack(outs, axis=0)


# revision 21
# speedup vs baseline: 2.9197x; 1.1757x over previous
"""AttentionDownSample Trainium2 kernel (8 NeuronCores, data-parallel over batch).

Reference computation per batch b (fm [C=128, H=256, W=256], d=2):
  window tokens x_t[c, oh, ow] = fm[c, 2*oh+dy, 2*ow+dx], t = dy*2+dx
  q  = mean_t x_t                      -> q_proj = (q @ Wq) * Cr^-0.5
  k_t = x_t @ Wk
  l_t = q_proj . k_t  (over Cr=32)
  a_t = softmax_t(l_t)
  out[c, oh, ow] = sum_t a_t * x_t[c, oh, ow]

Device strategy (per core, one batch):
  - DMA fm rows into SBUF in C-partition layout (even/odd row planes).
  - TensorE: lhsT = pixel slab slice [C, 128 positions] (stationary), rhs =
    [Wk | Wq_eff] -> k_t and accumulated q_proj land position-major in PSUM.
  - DVE/ACT: tiny per-position mul/reduce/exp/recip -> softmax weights w
    [128 pos, 4].
  - TensorE: transpose w (4 x 128 per row) and broadcast each w_t row across
    128 partitions with a K=1 rank-1 matmul (ones ⊗ w_t).
  - DVE: out = sum_t x_t * u_t  (4 mul + 3 add), DMA out.
"""

import os
import sys

sys.path.insert(0, "/opt/trn_rl_repo")

import numpy as np

B, C, H, W = 8, 128, 256, 256
CR = 32
NH, NW = H // 2, W // 2
N_CORES = 8
ROWS_PER_CHUNK = 4  # output rows (oh) per chunk


def build_bass(n_oh_rows=NH):
    import concourse.bass as bass
    import concourse.mybir as mybir
    from concourse import bacc, tile

    f32 = mybir.dt.float32
    nc = bacc.Bacc()

    n_h = n_oh_rows * 2
    fm = nc.declare_dram_parameter("fm", [C, n_h, W], f32, isOutput=False)
    wqk = nc.declare_dram_parameter("wqk", [C, 2 * CR], f32, isOutput=False)
    ident = nc.declare_dram_parameter("ident", [128, 128], f32, isOutput=False)
    # sel[k, t, m] = 1.0 iff k == t ; lhsT selector for row-broadcast matmuls
    sel = nc.declare_dram_parameter("sel", [4, 4, 128], f32, isOutput=False)
    out = nc.declare_dram_parameter("out", [C, n_oh_rows, NW], f32, isOutput=True)

    n_chunks = n_oh_rows // ROWS_PER_CHUNK
    R = ROWS_PER_CHUNK
    bf16 = mybir.dt.bfloat16

    with tile.TileContext(nc) as tc:
        with (
            tc.tile_pool(name="const", bufs=1) as cpool,
            tc.tile_pool(name="io", bufs=3) as iopool,
            tc.tile_pool(name="work", bufs=2) as wpool,
            tc.tile_pool(name="psum", bufs=1, space="PSUM") as ppool,
            tc.tile_pool(name="psumk", bufs=2, space="PSUM") as ppoolk,
        ):
            wqk_s = cpool.tile([C, 2 * CR], bf16)
            nc.gpsimd.dma_start(wqk_s[:], wqk[:])
            ident_s = cpool.tile([128, 128], bf16)
            nc.gpsimd.dma_start(ident_s[:], ident[:])
            sel_s = cpool.tile([4, 4, 128], bf16)
            nc.gpsimd.dma_start(sel_s[:], sel[:])

            for ch in range(n_chunks):
                h0 = ch * 2 * R
                # even / odd source rows for this chunk, cast f32->bf16 in DMA
                fm_e = iopool.tile([C, R, W], bf16, tag="fm_e")
                fm_o = iopool.tile([C, R, W], bf16, tag="fm_o")
                nc.gpsimd.dma_start(fm_e[:], fm[:, h0 : h0 + 2 * R : 2, :])
                nc.gpsimd.dma_start(fm_o[:], fm[:, h0 + 1 : h0 + 2 * R : 2, :])

                # ---- projections: k_t and q_proj, position-major ----
                # psum_k[pos, r, t, cr]; psum_q[pos, r, cr]
                psum_k = ppoolk.tile([128, R, 4, CR], f32, tag="pk")
                psum_q = ppool.tile([128, R, CR], f32, tag="pq")
                for r in range(R):
                    for t in range(4):
                        dy, dx = t // 2, t % 2
                        src = fm_e if dy == 0 else fm_o
                        xsl = src[:, r, dx::2]  # [128, 128] strided
                        nc.tensor.matmul(
                            psum_k[:, r, t, :], xsl, wqk_s[:, 0:CR],
                            start=True, stop=True,
                        )
                        nc.tensor.matmul(
                            psum_q[:, r, :], xsl, wqk_s[:, CR : 2 * CR],
                            start=(t == 0), stop=(t == 3),
                        )

                # ---- logits + softmax (position-major, DVE/ACT) ----
                qs = wpool.tile([128, R, CR], f32, tag="qs")
                nc.scalar.copy(qs[:], psum_q[:])
                prod = wpool.tile([128, R, 4, CR], f32, tag="prod")
                _q = qs[:]
                qs_b = bass.AP(_q.tensor, _q.offset, _q.ap[:2] + [[0, 4]] + _q.ap[2:])
                nc.vector.tensor_tensor(
                    prod[:], psum_k[:], qs_b, mybir.AluOpType.mult
                )
                logit = wpool.tile([128, R, 4], f32, tag="logit")
                nc.vector.tensor_reduce(
                    logit[:], prod[:], mybir.AxisListType.X, mybir.AluOpType.add
                )
                el = wpool.tile([128, R, 4], f32, tag="el")
                nc.scalar.activation(
                    el[:], logit[:], mybir.ActivationFunctionType.Exp
                )
                zsum = wpool.tile([128, R], f32, tag="zsum")
                nc.vector.tensor_reduce(
                    zsum[:], el[:], mybir.AxisListType.X, mybir.AluOpType.add
                )
                rz = wpool.tile([128, R], f32, tag="rz")
                nc.vector.reciprocal(rz[:], zsum[:])
                wgt = wpool.tile([128, R, 4], bf16, tag="wgt")
                _rz = rz[:]
                rz_b = bass.AP(_rz.tensor, _rz.offset, _rz.ap + [[0, 4]])
                nc.vector.tensor_tensor(
                    wgt[:], el[:], rz_b, mybir.AluOpType.mult
                )

                # ---- transpose w and broadcast across partitions ----
                psum_wt = ppool.tile([4, R, 128], bf16, tag="pwt")
                for r in range(R):
                    nc.tensor.transpose(
                        psum_wt[:, r, :], wgt[:, r, :], ident_s[:]
                    )
                wts = wpool.tile([4, R, 128], bf16, tag="wts")
                nc.scalar.copy(wts[:], psum_wt[:])

                psum_u = ppool.tile([128, 4, R, 128], f32, tag="pu")
                for t in range(4):
                    nc.tensor.matmul(
                        psum_u[:, t], sel_s[:, t, :],
                        wts[:].rearrange("k r m -> k (r m)"),
                        start=True, stop=True,
                    )

                # copy PSUM->SBUF bf16 with interleaving dest AP:
                # ui[p, dy, r, 2*ow+dx] = w_{2dy+dx}[r, ow]
                ui = wpool.tile([128, 2, R, W], bf16, tag="ui")
                for t in range(4):
                    dy, dx = t // 2, t % 2
                    nc.scalar.copy(ui[:, dy, :, dx::2], psum_u[:, t])

                # ---- weighted sum: all contiguous bf16 (2x DVE mode) ----
                ve = wpool.tile([128, R, W], bf16, tag="ve")
                vo = wpool.tile([128, R, W], bf16, tag="vo")
                nc.vector.tensor_tensor(
                    ve[:], fm_e[:], ui[:, 0], mybir.AluOpType.mult
                )
                nc.vector.tensor_tensor(
                    vo[:], fm_o[:], ui[:, 1], mybir.AluOpType.mult
                )
                vs = wpool.tile([128, R, W], bf16, tag="vs")
                nc.vector.tensor_tensor(vs[:], ve[:], vo[:], mybir.AluOpType.add)
                acc = wpool.tile([128, R, 128], f32, tag="acc")
                nc.gpsimd.tensor_tensor(
                    acc[:], vs[:, :, 0::2], vs[:, :, 1::2], mybir.AluOpType.add
                )

                nc.sync.dma_start(out[:, ch * R : (ch + 1) * R, :], acc[:])

    nc.compile()
    return nc


_NC_CACHE = {}


def _get_nc(n_oh_rows=NH):
    if n_oh_rows not in _NC_CACHE:
        _NC_CACHE[n_oh_rows] = build_bass(n_oh_rows)
    return _NC_CACHE[n_oh_rows]


def _make_in_maps(fm, Wq, Wk):
    wq_eff = (Wq.astype(np.float64) * (CR ** -0.5) / 4.0).astype(np.float32)
    wqk = np.concatenate([Wk.astype(np.float32), wq_eff], axis=1)
    wqk = np.ascontiguousarray(wqk)
    ident = np.eye(128, dtype=np.float32)
    sel = np.zeros((4, 4, 128), dtype=np.float32)
    for t in range(4):
        sel[t, t, :] = 1.0
    return [
        {
            "fm": np.ascontiguousarray(fm[i]),
            "wqk": wqk,
            "ident": ident,
            "sel": sel,
        }
        for i in range(fm.shape[0])
    ]


def kernel(fm, Wq, Wk):
    from concourse.bass_utils import run_bass_kernel_spmd

    fm = np.asarray(fm, dtype=np.float32)
    Wq = np.asarray(Wq, dtype=np.float32)
    Wk = np.asarray(Wk, dtype=np.float32)

    nc = _get_nc()
    in_maps = _make_in_maps(fm, Wq, Wk)
    res = run_bass_kernel_spmd(nc, in_maps, core_ids=list(range(N_CORES)))
    outs = [np.asarray(res.results[i]["out"]) for i in range(N_CORES)]
    return np.stack(outs, axis=0)
